# revision 113
# baseline (speedup 1.0000x reference)
"""Multi-head self-attention (N=2, S=2048, E=1024, 16 heads) on 8 trn2 cores.

Sharding: data parallel over batch (2) x tensor parallel over heads (4 groups
of 4 heads). Each core computes in_proj for its local heads, attention with
full SxS scores for its local heads, and a partial out_proj (contraction over
its local 256 features). Host sums the 4 partials per batch and adds b_o.

Per-core kernel (v5, software-pipelined, ACT(exp)-bound):
  - in_proj: x/w in bf16, psum f32; q/k biased into fp8e4 tiles
    [128, 2ch, 2i, S] (features on partitions). The i axis feeds DoubleRow
    score matmuls: q carries (q8, residual q-q8) so the pair sums to ~exact
    q, k duplicates its fp8 value into both planes. V in bf16
    [tok, head, dim] with a memset ones column (denominators via matmul).
  - scores per 128-key tile: one fp8 DoubleRow matmul per (hh, 256q):
    stationary k8 [64, 2, 128], moving q8 [64, 2, 256] at 0.5 cyc/col —
    half the PE time of the bf16/f32r equivalent. exp on ACT into bf16
    ex tiles [128 keys, 2hh, W] (the 83%-busy bottleneck engine).
  - attnV per (kt, hh, 128q): stationary ex, moving V||ones [128, 65]
    bf16 into psum accumulators [128q, 4qq, 65] per head.
  - divide by denominator: per-partition reciprocal + tensor_scalar_mul
    into bf16 oT tiles; out_proj per 128-query block (PE transpose + two
    512-wide matmuls); bf16 partials leave in one DMA per block (the last
    two blocks split per-half so the final transfer starts early).
  - the last query block runs as two 256-query sub-phases so its epilogue
    (divide/transpose/out_proj/DMA) is half as deep.
  Scheduling: scores+exp for kt+1 are emitted (double look-ahead) before
  the deferred attnV/filler work of cycle kt, so the in-order PE stream and
  the coarse tick-based waits never park a filler ahead of the next exp.
  in_proj runs as 128-token half-tiles (426ns, fits a 1038ns exp cycle's
  PE slack) on a deadline schedule matched to DMA arrival and phase needs;
  attnVs are gated on the previous phase's div (psO WAR) so blocked matmuls
  never jam the PE engine queue; dummy matmuls warm the PE p-state ramp.
"""
import collections
import os

import numpy as np

import concourse.bacc as bacc
import concourse.mybir as mybir
from concourse.tile import TileContext
from concourse.bass import ts

F32 = mybir.dt.float32
F32R = mybir.dt.float32r
BF16 = mybir.dt.bfloat16
F8 = mybir.dt.float8e4
EXP = mybir.ActivationFunctionType.Exp
IDN = mybir.ActivationFunctionType.Identity
CPY = mybir.ActivationFunctionType.Copy
DROW = mybir.MatmulPerfMode.DoubleRow

D_MODEL = 1024
NHEAD = 16
DH = 64
N_BATCH = 2
SEQ = 2048
N_CORES = 8
GROUPS = 4            # head groups (cores per batch)
HL = NHEAD // GROUPS  # local heads per core = 4
FL = HL * DH          # local feature width = 256

N_DUMMY = 13          # PE warm-up matmuls while the first DMAs land
DBG_STAGE = int(os.environ.get("KDBG", "9"))
DBG_POQ = int(os.environ.get("KPOQ", "99"))  # 1=inproj 2=+scores/exp 3=+attnv 4=+div 5=+tp 9=full


def build_mha(nc, S=SEQ, E=D_MODEL, EOUT=D_MODEL, scale=0.125):
    FLOC = FL                 # local q/k/v feature count (256)
    EC = E // 128             # contraction chunks for in_proj (8)
    TT = S // 128             # token tiles (16)
    KT = S // 128             # 128-wide key tiles (16)

    xT = nc.dram_tensor("xT", [E, S], BF16, kind="ExternalInput")
    wT = nc.dram_tensor("wT", [E, 3 * FLOC], BF16, kind="ExternalInput")
    qkb = nc.dram_tensor("qkb", [128, 4], F32, kind="ExternalInput")
    vbr = nc.dram_tensor("vbr", [128, HL, DH], BF16, kind="ExternalInput")
    woT = nc.dram_tensor("woT", [FLOC, EOUT], BF16, kind="ExternalInput")
    ident = nc.dram_tensor("ident", [128, 128], BF16, kind="ExternalInput")
    out = nc.dram_tensor("out", [S, EOUT], BF16, kind="ExternalOutput")

    with TileContext(nc) as tc:
        with tc.tile_pool(name="pp", bufs=1) as pp, \
             tc.tile_pool(name="pw", bufs=1) as pw, \
             tc.tile_pool(name="psS", bufs=2, space="PSUM") as psS, \
             tc.tile_pool(name="psO", bufs=1, space="PSUM") as psO, \
             tc.tile_pool(name="psM", bufs=2, space="PSUM") as psM:
            xT_sb = pp.tile([128, EC, S], BF16)
            wT_sb = pp.tile([128, EC, 3 * FLOC], BF16)
            # fp8 q/k for DoubleRow score matmuls (0.5 cyc/col): the sum
            # over i of k8[:,ch,i,:].T @ q8[:,ch,i,:] pairs k (duplicated
            # into both i planes) with (q8, q-q8 residual), recovering
            # near-bf16 accuracy on the q side at fp8 matmul cost.
            qT = pp.tile([128, 2, 2, S], F8)
            kT = pp.tile([128, 2, 2, S], F8)
            v = pp.tile([128, TT, HL, 65], BF16)
            woT_sb = pp.tile([128, 2, EOUT], BF16)
            qkb_sb = pp.tile([128, 4], F32)
            vbr_sb = pp.tile([128, HL, DH], BF16)
            ident_sb = pp.tile([128, 128], BF16)
            dums = pp.tile([128, 512], BF16)

            # ---- DMA issue order = data priority (single SP queue, so
            # completion order == issue order and PE waits stay in dispatch
            # order). First-exp deps (w-ch0, x0, qkb, x1) go first.
            xTr = xT.rearrange("(c p) s -> p c s", p=128)
            wTr = wT.rearrange("(c p) f -> p c f", p=128)
            # wT column layout: [q-ch0 | k-ch0 | q-ch1 | k-ch1 | v] so the
            # first-exp weights (q-ch0|k-ch0) land in ONE contiguous DMA.
            nc.sync.dma_start(wT_sb[:, :, 0:256], wTr[:, :, 0:256])  # q|k ch0
            # first two x chunks split by contraction-halves: the prelude's
            # first 4-chunk accumulation groups start one half-transfer in
            nc.sync.dma_start(xT_sb[:, 0:4, 0:256], xTr[:, 0:4, 0:256])
            nc.sync.dma_start(xT_sb[:, 4:8, 0:256], xTr[:, 4:8, 0:256])
            nc.sync.dma_start(qkb_sb[:], qkb[:])
            nc.sync.dma_start(xT_sb[:, 0:4, 256:512], xTr[:, 0:4, 256:512])
            nc.sync.dma_start(xT_sb[:, 4:8, 256:512], xTr[:, 4:8, 256:512])
            for t0 in (512, 768, 1024):
                nc.sync.dma_start(xT_sb[:, :, t0:t0 + 256], xTr[:, :, t0:t0 + 256])
            nc.sync.dma_start(wT_sb[:, :, 512:768], wTr[:, :, 512:768])  # v
            nc.sync.dma_start(xT_sb[:, :, 1280:1536], xTr[:, :, 1280:1536])
            nc.sync.dma_start(vbr_sb[:], vbr[:])
            for t0 in (1536, 1792):
                nc.sync.dma_start(xT_sb[:, :, t0:t0 + 256], xTr[:, :, t0:t0 + 256])
            nc.sync.dma_start(wT_sb[:, :, 256:512], wTr[:, :, 256:512])  # q|k ch1
            nc.sync.dma_start(ident_sb[:], ident[:])
            nc.sync.dma_start(woT_sb[:], woT.rearrange("(c p) e -> p c e", p=128))

            # ---- PE p-state warm-up on zeroed scratch ----
            nc.vector.memset(dums[:], 0.0)
            for _ in range(N_DUMMY):
                pm = psM.tile([128, 512], F32, tag="m", name="pdum")
                nc.tensor.matmul(pm[:, 0:256], dums[:, 0:128], dums[:, 0:256],
                                 start=True, stop=True)

            # ones column for the softmax denominators (cheap on DVE; a DMA
            # of 1-byte elements costs ~3.6us of descriptor time)
            nc.vector.memset(v[:, :, :, 64:65], 1.0)

            # ---- in_proj work units ----
            cyc = [0]        # current global kt-cycle (mutable for closures)
            v_done = [None] * TT

            def qk_tile(dst, ft, wcol, bi, t0, is_q, W=256):
                def emit():
                    pm = psM.tile([128, 512], F32, tag="m", name="pqk")
                    for c in range(EC):
                        nc.tensor.matmul(pm[:, 0:W], wT_sb[:, c, wcol:wcol + 128],
                                         xT_sb[:, c, t0:t0 + W],
                                         start=(c == 0), stop=(c == EC - 1))
                    nc.vector.tensor_scalar_add(dst[:, ft, 0, t0:t0 + W],
                                                pm[:, 0:W],
                                                qkb_sb[:, bi:bi + 1])
                    if is_q:
                        # i=1 plane: fp8 residual (pm+b) - fp8(pm+b), so the
                        # DoubleRow sum k.(q8+qr8) cancels q's fp8 error
                        nc.vector.scalar_tensor_tensor(
                            dst[:, ft, 1, t0:t0 + W], pm[:, 0:W],
                            qkb_sb[:, bi:bi + 1], dst[:, ft, 0, t0:t0 + W],
                            mybir.AluOpType.add, mybir.AluOpType.subtract)
                    else:
                        # k duplicated into both i-planes (pairs q8 and qr8)
                        nc.vector.tensor_copy(dst[:, ft, 1, t0:t0 + W],
                                              dst[:, ft, 0, t0:t0 + W])
                return emit

            def v_tile(t, half=None):
                def emit():
                    pm = psM.tile([128, 512], F32, tag="m", name="pv")
                    if half is None:
                        f0, fw = 0, 256
                    else:
                        f0, fw = 128 * half, 128
                    for c in range(EC):
                        nc.tensor.matmul(pm[:, 0:fw], xT_sb[:, c, ts(t, 128)],
                                         wT_sb[:, c,
                                               2 * FLOC + f0:2 * FLOC + f0 + fw],
                                         start=(c == 0), stop=(c == EC - 1))
                    h0 = f0 // 64
                    nh = fw // 64
                    nc.vector.tensor_add(
                        v[:, t, h0:h0 + nh, 0:64],
                        pm[:, 0:fw].rearrange("p (h d) -> p h d", h=nh),
                        vbr_sb[:, h0:h0 + nh])
                    if half is None or half == 1:
                        v_done[t] = cyc[0]
                return emit

            k0 = lambda t0: qk_tile(kT, 0, 128, 2, t0, False)
            k1 = lambda t0: qk_tile(kT, 1, 384, 3, t0, False)
            q0 = lambda t0: qk_tile(qT, 0, 0, 0, t0, True)
            q1 = lambda t0: qk_tile(qT, 1, 256, 1, t0, True)
            # 128-token halves: one fits a steady-state cycle's PE slack
            # (426ns) where a 256-token tile (856ns) overshoots and slips exp
            k0h = lambda t0: qk_tile(kT, 0, 128, 2, t0, False, W=128)
            k1h = lambda t0: qk_tile(kT, 1, 384, 3, t0, False, W=128)
            q0h = lambda t0: qk_tile(qT, 0, 0, 0, t0, True, W=128)
            q1h = lambda t0: qk_tile(qT, 1, 256, 1, t0, True, W=128)

            # Eager prelude: deps of the first exps (kt 0..3 of the
            # queries-0:512 phase): q0 over 0:512, k0 covering keys 0:512.
            # k0(0)/q0(0) interleave their 4-chunk accumulation halves so PE
            # starts on the first half-transfer of x0 and never stalls.
            pmK = psM.tile([128, 512], F32, tag="m", name="pqk")
            pmQ = psM.tile([128, 512], F32, tag="m", name="pqk")
            for pm, wc in ((pmK, 128), (pmQ, 0)):
                for c in range(4):
                    nc.tensor.matmul(pm[:, 0:256], wT_sb[:, c, wc:wc + 128],
                                     xT_sb[:, c, 0:256],
                                     start=(c == 0), stop=False)
            for pm, wc in ((pmK, 128), (pmQ, 0)):
                for c in range(4, 8):
                    nc.tensor.matmul(pm[:, 0:256], wT_sb[:, c, wc:wc + 128],
                                     xT_sb[:, c, 0:256],
                                     start=False, stop=(c == 7))
            nc.vector.tensor_scalar_add(kT[:, 0, 0, 0:256], pmK[:, 0:256],
                                        qkb_sb[:, 2:3])
            nc.vector.tensor_copy(kT[:, 0, 1, 0:256], kT[:, 0, 0, 0:256])
            nc.vector.tensor_scalar_add(qT[:, 0, 0, 0:256], pmQ[:, 0:256],
                                        qkb_sb[:, 0:1])
            nc.vector.scalar_tensor_tensor(
                qT[:, 0, 1, 0:256], pmQ[:, 0:256], qkb_sb[:, 0:1],
                qT[:, 0, 0, 0:256],
                mybir.AluOpType.add, mybir.AluOpType.subtract)
            q0(256)()
            k0(256)()

            # Cycle-scheduled fillers, matched to DMA arrival and phase
            # deadlines (hp-outer: kT ch0 + q0@512:1024 by c14, v by ~c22,
            # ch1 by c62+, late q1 feeds the PE-light late phases).
            fs = []
            # k-ch0 keys 512:2048 + q-ch0 512:1024 as halves, c1-12
            for i, t0 in enumerate(range(512, 2048, 128)):
                fs.append((1 + i, k0h(t0)))
            for i, t0 in enumerate(range(512, 1024, 128)):
                fs.append((3 + 2 * i, q0h(t0)))
            # v halves 1/cycle; q0-late halves in dedicated slots (deadline:
            # phase 2 scores at c31, phase 3 at c47)
            vh = [v_tile(t, h) for t in range(TT) for h in range(2)]
            vslots = (list(range(13, 23)) + list(range(27, 37)) +
                      list(range(41, 53)))
            for c, f in zip(vslots, vh):
                fs.append((c, f))
            for i, t0 in enumerate(range(1024, 1536, 128)):
                fs.append((23 + i, q0h(t0)))
            for i, t0 in enumerate(range(1536, 2048, 128)):
                fs.append((37 + i, q0h(t0)))
            # ch1: q1 0:512 doubled into the v tail (needed c64), k1 halves
            # 1/cycle c53-68 (phase-4 kt pace), late q1 on phase deadlines
            for i, t0 in enumerate(range(0, 512, 128)):
                fs.append((42 + 2 * i, q1h(t0)))
            for i, t0 in enumerate(range(0, 2048, 128)):
                fs.append((53 + i, k1h(t0)))
            for i, t0 in enumerate(range(512, 1024, 128)):
                fs.append((70 + 2 * i, q1h(t0)))
            for i, t0 in enumerate(range(1024, 1536, 128)):
                fs.append((82 + 3 * i, q1h(t0)))
            for i, t0 in enumerate(range(1536, 2048, 128)):
                fs.append((98 + 3 * i, q1h(t0)))
            fsched = collections.deque(fs)

            fsched = collections.deque(sorted(fsched, key=lambda x: x[0]))

            def run_fillers(maxn=2):
                n = 0
                while fsched and fsched[0][0] <= cyc[0] and n < maxn:
                    fsched.popleft()[1]()
                    n += 1

            # ---- attention pipeline state ----
            # phases: (hp, q0_off, width)
            phases = [(0, 0, 512), (0, 512, 512), (0, 1024, 512),
                      (0, 1536, 512),
                      (1, 0, 512), (1, 512, 512), (1, 1024, 512),
                      (1, 1536, 256), (1, 1792, 256)]
            NP = len(phases)
            ex_store = {}
            oacc = {}        # phase idx -> (oa, ob)
            oT_tiles = {}    # global 128-query tile index tq -> sbuf tile
            osb_tiles = {}
            pend = collections.deque()  # (kind, payload, enq_cycle)
            pstart = [0]     # cycle at which the current phase started

            def emit_attnv(P, kt):
                hp, off, W = phases[P]
                oa, ob = oacc[P]
                ex = ex_store.pop((P, kt))
                nq = W // 128
                for hh, acc in ((0, oa), (1, ob)):
                    for qq in range(nq):
                        # one accumulation group per psum BANK (2KB zero
                        # region): start only on the bank's first write,
                        # stop only on its last
                        nc.tensor.matmul(
                            acc[:, qq, :],
                            ex[:, hh, ts(qq, 128)],
                            v[:, kt, 2 * hp + hh, :],
                            start=(kt == 0 and qq == 0),
                            stop=(kt == KT - 1 and qq == nq - 1))

            def emit_recs(P, oa, ob):
                if DBG_STAGE < 4:
                    return [None, None]
                hp, off, W = phases[P]
                nq = W // 128
                recs = []
                for acc in (oa, ob):
                    rec = pw.tile([128, 4], F32, tag="rec", bufs=4,
                                  name="rec")
                    nc.vector.reciprocal(
                        rec[:, 0:nq],
                        acc[:, 0:nq, 64:65].rearrange("p q one -> p (q one)"))
                    recs.append(rec)
                return recs

            def emit_div_qq(P, qq, oa, ob, recs, act=False):
                if DBG_STAGE < 4:
                    return
                hp, off, W = phases[P]
                tq = off // 128 + qq
                if tq not in oT_tiles:
                    oT_tiles[tq] = pw.tile([128, 256], BF16, tag="ot",
                                           bufs=16, name="oT")
                oT = oT_tiles[tq]
                for hh, acc, rec in ((0, oa, recs[0]), (1, ob, recs[1])):
                    off2 = (2 * hp + hh) * 64
                    if act:
                        # final epilogue: ACT is idle, do x*rec there
                        nc.scalar.activation(
                            oT[:, off2:off2 + 64], acc[:, qq, 0:64],
                            CPY, scale=rec[:, qq:qq + 1])
                    else:
                        nc.vector.tensor_scalar_mul(
                            oT[:, off2:off2 + 64], acc[:, qq, 0:64],
                            rec[:, qq:qq + 1])

            def emit_div(P):
                hp, off, W = phases[P]
                oa, ob = oacc.pop(P)
                recs = emit_recs(P, oa, ob)
                for qq in range(W // 128):
                    emit_div_qq(P, qq, oa, ob, recs)

            def emit_tp(tq, pool=False):
                if DBG_STAGE < 5 or tq not in oT_tiles:
                    return
                oT = oT_tiles[tq]
                osb = pw.tile([128, 2, 128], BF16, tag="osb", bufs=5,
                              name="osb")
                # one full psum bank per transpose: a second is_transpose
                # matmul at a 256B psum offset faults the exec unit
                for c in range(2):
                    tp = psM.tile([128, 128], BF16, tag="m", name="tp")
                    nc.tensor.transpose(tp[:], oT[:, ts(c, 128)], ident_sb[:])
                    nc.vector.tensor_copy(osb[:, c, :], tp[:])
                osb_tiles[tq] = osb

            def emit_po(tq, act=False):
                if DBG_STAGE < 6 or tq not in osb_tiles or tq >= DBG_POQ:
                    return
                del oT_tiles[tq]
                osb = osb_tiles.pop(tq)
                tail = tq >= TT - 1
                fo = pw.tile([128, 1024], BF16, tag="fo", bufs=6, name="fo")
                for eb in range(2):
                    pm = psM.tile([128, 512], F32, tag="m", name="po")
                    for c in range(2):
                        nc.tensor.matmul(pm[:], osb[:, c, :],
                                         woT_sb[:, c, ts(eb, 512)],
                                         start=(c == 0), stop=(c == 1))
                    if tq >= TT - 2 and eb == 0:
                        # tail: first half converts on the idle ACT engine,
                        # in parallel with the second half's matmuls + DVE
                        nc.scalar.activation(fo[:, ts(eb, 512)], pm[:], CPY)
                    else:
                        nc.vector.tensor_copy(fo[:, ts(eb, 512)], pm[:])
                    # tail tiles: DMA each half right after its copy so the
                    # last transfer isn't serialized behind both copies
                    if tail and DBG_STAGE >= 9:
                        nc.sync.dma_start(out[ts(tq, 128), ts(eb, 512)],
                                          fo[:, ts(eb, 512)])
                if not tail and DBG_STAGE >= 9:
                    nc.sync.dma_start(out[ts(tq, 128), :], fo[:])

            last_phase = [False]

            div_cycle = {}   # phase -> cycle its div drained

            def ready(item):
                kind, payload, enq = item
                if kind == "attnv":
                    P, kt = payload
                    # oa(P) allocation WARs on div(P-1)'s DVE reads; keep
                    # attnvs out of the in-order PE stream until that div
                    # has drained, else >4 blocked attnvs jam the engine
                    # wait queue and stall even ready scores behind them.
                    if P > 0 and not (div_cycle.get(P - 1, 99999) + 1
                                      <= cyc[0]):
                        return False
                    lag = 1 if last_phase[0] else 2
                    if enq > cyc[0] - lag:
                        return False
                    return v_done[kt] is not None and v_done[kt] < cyc[0]
                if kind == "div":
                    return enq < cyc[0]
                if kind == "tp" or kind == "po":
                    # keep phase starts clear for scores so ACT never starves
                    if cyc[0] - pstart[0] < (8 if payload >= 12 else 16):
                        return False
                    return enq < cyc[0] if kind == "tp" else enq <= cyc[0] - 2
                raise AssertionError(kind)

            def drain_pend(maxn):
                n = 0
                while pend and n < maxn:
                    item = pend[0]
                    if not ready(item):
                        break
                    pend.popleft()
                    kind, payload, _ = item
                    if kind == "attnv":
                        emit_attnv(*payload)
                    elif kind == "div":
                        emit_div(payload)
                        div_cycle[payload] = cyc[0]
                    elif kind == "tp":
                        emit_tp(payload)
                    elif kind == "po":
                        emit_po(payload)
                    n += 1

            def emit_scores(hp, off, W, kt):
                sps = psS.tile([128, 2, 512], F32, tag="s", name="sps")
                for hh in range(2):
                    p0 = 64 * hh
                    for qh in range(0, W, 256):
                        qw = min(256, W - qh)
                        nc.tensor.matmul(
                            sps[:, hh, qh:qh + qw],
                            kT[p0:p0 + 64, hp, :, ts(kt, 128)],
                            qT[p0:p0 + 64, hp, :,
                               off + qh:off + qh + qw],
                            start=True, stop=True, perf_mode=DROW)
                return sps

            # ---- main attention loop (head-pair outer, query-block inner).
            # Scores for kt+1 are emitted right after exp(kt), BEFORE the
            # deferred attnV/fillers, so an in-flight filler (850ns of PE)
            # never sits ahead of the next exp's scores in the in-order PE
            # stream (that ordering starves ACT ~200ns per kt).
            def emit_exp(key, W, sps):
                ex = pw.tile([128, 2, 512], BF16, tag="ex", bufs=40,
                             name="ex")
                ex_store[key] = ex
                nc.scalar.activation(ex[:, :, 0:W], sps[:, :, 0:W],
                                     EXP, scale=scale)

            # double look-ahead: scores AND exp for kt+1 (across phase
            # boundaries too) are emitted before this cycle's deferred
            # attnV/filler work, so the coarse tick-based waits the sem
            # assigner gives the exp never cover a filler.
            sps = emit_scores(*phases[0][:3], 0)
            emit_exp((0, 0), phases[0][2], sps)
            for P, (hp, off, W) in enumerate(phases):
                last_phase[0] = P == NP - 1
                pstart[0] = cyc[0]
                oa = psO.tile([128, 4, 65], F32, tag="oa", name="oa")
                ob = psO.tile([128, 4, 65], F32, tag="ob", name="ob")
                oacc[P] = (oa, ob)
                for kt in range(KT):
                    if kt + 1 < KT:
                        sps = emit_scores(hp, off, W, kt + 1)
                        emit_exp((P, kt + 1), W, sps)
                    elif P + 1 < NP:
                        sps = emit_scores(*phases[P + 1][:3], 0)
                        emit_exp((P + 1, 0), phases[P + 1][2], sps)
                    pend.append(("attnv", (P, kt), cyc[0]))
                    cyc[0] += 1
                    drain_pend(6)
                    run_fillers()
                if P < NP - 1:
                    pend.append(("div", P, cyc[0]))
                    if hp == 1:
                        for qq in range(W // 128):
                            tq = off // 128 + qq
                            pend.append(("tp", tq, cyc[0] + qq))
                            pend.append(("po", tq, cyc[0] + qq))
            # ---- epilogue: pipelined finish of the last sub-phase ----
            while pend:
                item = pend.popleft()
                kind, payload, _ = item
                if kind == "attnv":
                    emit_attnv(*payload)
                elif kind == "div":
                    emit_div(payload)
                elif kind == "tp":
                    emit_tp(payload)
                elif kind == "po":
                    emit_po(payload)
            P = NP - 1
            hp, off, W = phases[P]
            oa, ob = oacc.pop(P)
            recs = emit_recs(P, oa, ob)
            # stage-batched: both divs, then both transposes, then both
            # out_projs — the two query-tiles' chains pipeline across engines
            for qq in range(W // 128):
                emit_div_qq(P, qq, oa, ob, recs)
            for qq in range(W // 128):
                emit_tp(off // 128 + qq)
            for qq in range(W // 128):
                if DBG_STAGE >= 5:
                    emit_po(off // 128 + qq)
            while fsched:
                fsched.popleft()[1]()


_CACHED = {}


def _get_module():
    if "nc" not in _CACHED:
        nc = bacc.Bacc("TRN2")
        build_mha(nc)
        nc.finalize()
        _CACHED["nc"] = nc
    return _CACHED["nc"]


def make_in_maps(query, w_in, b_in, w_o):
    """Host-side sharding: per-core input dicts (layout transforms included)."""
    import ml_dtypes
    BF = ml_dtypes.bfloat16
    E, FLoc = D_MODEL, FL
    woT_full = np.ascontiguousarray(w_o.T, dtype=np.float32)  # (e_in, e_out)
    ident_arr = np.eye(128, dtype=BF)
    in_maps = []
    for core in range(N_CORES):
        b, g = divmod(core, GROUPS)
        rows = np.r_[g * FLoc:(g + 1) * FLoc,
                     E + g * FLoc:E + (g + 1) * FLoc,
                     2 * E + g * FLoc:2 * E + (g + 1) * FLoc]
        bl = b_in[rows].astype(np.float32)
        # qkb columns: q-ch0, q-ch1, k-ch0, k-ch1
        qkb_c = np.ascontiguousarray(
            np.stack([bl[0:128], bl[128:256], bl[256:384], bl[384:512]],
                     axis=1).astype(np.float32))
        vbr_c = np.ascontiguousarray(
            np.broadcast_to(bl[2 * FLoc:].reshape(1, HL, DH),
                            (128, HL, DH))).astype(BF)
        wTh = w_in[rows].T  # (E, 768) cols [q(256) | k(256) | v(256)]
        # kernel column order: [q-ch0 | k-ch0 | q-ch1 | k-ch1 | v]
        wTh = np.concatenate([wTh[:, 0:128], wTh[:, 256:384],
                              wTh[:, 128:256], wTh[:, 384:512],
                              wTh[:, 512:768]], axis=1)
        in_maps.append({
            "xT": np.ascontiguousarray(query[b].T).astype(BF),
            "wT": np.ascontiguousarray(wTh).astype(BF),
            "qkb": qkb_c,
            "vbr": vbr_c,
            "woT": np.ascontiguousarray(woT_full[g * FLoc:(g + 1) * FLoc]).astype(BF),
            "ident": ident_arr,
        })
    return in_maps


def kernel(query, key, value, w_in, b_in, w_o, b_o, _trace=False):
    from concourse.bass_utils import run_bass_kernel_spmd
    query = np.asarray(query, dtype=np.float32)
    nc = _get_module()
    in_maps = make_in_maps(query, np.asarray(w_in), np.asarray(b_in),
                           np.asarray(w_o))
    res = run_bass_kernel_spmd(nc, in_maps, core_ids=list(range(N_CORES)),
                               trace=_trace)
    out = np.empty((N_BATCH, SEQ, D_MODEL), np.float32)
    for b in range(N_BATCH):
        acc = res.results[b * GROUPS]["out"].astype(np.float32)
        for g in range(1, GROUPS):
            acc = acc + res.results[b * GROUPS + g]["out"]
        out[b] = acc + np.asarray(b_o, dtype=np.float32)[None, :]
    if _trace:
        kernel.last_exec_time_ns = res.exec_time_ns
    return out

